# revision 1
# baseline (speedup 1.0000x reference)
"""Causal single-head attention (B=4, S=2048, D=1024) on 8 Trainium2 cores.

Sharding: 2 cores per batch. Core parity p in {0,1} owns global query tiles
gq = 2t+p (t = 0..7), i.e. interleaved 128-query tiles. This makes the device
program identical on all 8 cores (only input data differs):
  - every core computes K^T and V projections for all 2048 keys of its batch
  - core p's t-th query tile attends (2t+2)*128 keys, with a parity-dependent
    additive mask input covering the last 256 key columns (causal diagonal)
Per-core phases:
  A1: K^T = W_K^T @ X^T       -> SBUF resident [128, 8, 2048] fp32r
  A2: V   = X @ W_V           -> SBUF resident [128, 16, 1024] fp32r
  A3: Q^T = (W_Q^T @ X_q^T)/32 -> DRAM scratch (SBUF too small in phase A)
  B:  flash attention per query tile: S = Q^T.T K^T (PSUM), +mask, exp (ACT,
      rowsum via accum), PE-transpose P, O += P^T.T V (PSUM), O *= 1/rowsum.
All matmuls run as fp32r (tf32-like, 1 cycle/row at N>=256), fp32 accumulate.
"""

import numpy as np

B, S, D = 4, 2048, 1024
NCORES = 8
DC = D // 128        # 8 contraction chunks
NKT = S // 128       # 16 key tiles
NQT = 8              # query tiles per core
SCALE = 1.0 / np.sqrt(np.float32(D))

_CACHE = {}


def _build(cfg=None):
    from contextlib import ExitStack

    from concourse import bacc
    import concourse.mybir as mybir
    import concourse.tile as tile

    cfg = cfg or {}
    WPOOL = cfg.get("wpool", 10)
    A2_COUTER = cfg.get("a2_couter", False)
    A3_COUTER = cfg.get("a3_couter", True)
    COPY_ALT = cfg.get("copy_alt", False)
    SWDGE_RT = cfg.get("swdge", False)
    V_COUTER = cfg.get("v_couter", False)

    FP32 = mybir.dt.float32
    FP32R = mybir.dt.float32r
    EXP = mybir.ActivationFunctionType.Exp
    COPY = mybir.ActivationFunctionType.Copy
    AX = mybir.AxisListType.X
    ADD = mybir.AluOpType.add

    nc = bacc.Bacc("TRN2", debug=False, num_devices=NCORES, dynamic_dma_scratch_size=4096)
    # X^T chunked layouts (host-prepared):
    #   xt   [kc, p, c, k]: 256-key chunks, all c contiguous per partition row
    #   xt512[kc, c, p, k]: 512-key chunks, per-c planes
    xt = nc.dram_tensor("xt", [8, 128, DC, 256], FP32R, kind="ExternalInput").ap()
    xt512 = nc.dram_tensor("xt512", [4, DC, 128, 512], FP32R, kind="ExternalInput").ap()
    xtq = nc.dram_tensor("xtq", [4, 128, DC, 256], FP32R, kind="ExternalInput").ap()
    wq = nc.dram_tensor("wq", [DC, 128, D], FP32R, kind="ExternalInput").ap()
    wk = nc.dram_tensor("wk", [DC, 128, D], FP32R, kind="ExternalInput").ap()
    wv = nc.dram_tensor("wv", [DC, 128, D], FP32R, kind="ExternalInput").ap()
    mask = nc.dram_tensor("mask", [128, 256], FP32, kind="ExternalInput").ap()
    ident_in = nc.dram_tensor("ident", [128, 128], FP32R, kind="ExternalInput").ap()
    o = nc.dram_tensor("o", [NQT, 128, D], FP32, kind="ExternalOutput").ap()

    with tile.TileContext(nc) as tc, ExitStack() as ctx:
        const = ctx.enter_context(tc.tile_pool(name="const", bufs=1))
        resident = ctx.enter_context(tc.tile_pool(name="resident", bufs=1))
        dram = ctx.enter_context(tc.tile_pool(name="dram", bufs=1, space="DRAM"))

        ident = const.tile([128, 128], FP32R)
        mask_sb = const.tile([128, 256], FP32)

        kt_sb = resident.tile([128, DC, S], FP32R)       # K^T [d | dc, keys]
        v_sb = resident.tile([128, NKT, D], FP32R)       # V   [k | ktile, dv]
        qt_dram = dram.tile([4, 128, DC, 256], FP32R)

        # ---------------- Phase A: projections ----------------
        with tc.tile_pool(name="wpool", bufs=WPOOL) as wp, \
             tc.tile_pool(name="xchS", bufs=cfg.get("xsbufs", 2)) as xsp, \
             tc.tile_pool(name="apsum", bufs=8, space="PSUM") as aps:

            def psum_copy(dst, src, i):
                if COPY_ALT and i % 2 == 1:
                    nc.scalar.copy(dst, src)
                else:
                    nc.vector.tensor_copy(dst, src)

            def kproj_256_couter(xch, kc256):
                """c-outer: wk[c] last-used at step c -> early release for wv loads."""
                kpss = [aps.tile([128, 512], FP32, tag="ps", name=f"kpo{m}") for m in range(DC)]
                for c in range(DC):
                    for m in range(DC):
                        nc.tensor.matmul(
                            kpss[m][:, 0:256],
                            wk_t[c][:, m * 128 : (m + 1) * 128],
                            xch[:, c, :],
                            start=(c == 0),
                            stop=(c == DC - 1),
                            skip_group_check=True,
                        )
                for m in range(DC):
                    psum_copy(
                        kt_sb[:, m, kc256 * 256 : (kc256 + 1) * 256], kpss[m][:, 0:256], m
                    )

            def kproj_256(xch, kc256):
                """K^T for one 256-key chunk held in [128, DC, 256] tile."""
                for m in range(DC):
                    kps = aps.tile([128, 512], FP32, tag="ps", name="kps")
                    for c in range(DC):
                        nc.tensor.matmul(
                            kps[:, 0:256],
                            wk_t[c][:, m * 128 : (m + 1) * 128],
                            xch[:, c, :],
                            start=(c == 0),
                            stop=(c == DC - 1),
                        )
                    psum_copy(
                        kt_sb[:, m, kc256 * 256 : (kc256 + 1) * 256], kps[:, 0:256], m
                    )

            def vproj_256(xch, kc256, couter=False):
                """V for one 256-key chunk held in [128, DC, 256] tile."""
                if couter:
                    vpss = [
                        aps.tile([128, 512], FP32, tag="ps", name=f"vps{j}{h}")
                        for j in range(2)
                        for h in range(2)
                    ]
                    for c in range(DC):
                        for jh in range(4):
                            nc.tensor.matmul(
                                vpss[jh][:],
                                xch[:, c, (jh // 2) * 128 : (jh // 2 + 1) * 128],
                                wv_t[c][:, (jh % 2) * 512 : (jh % 2 + 1) * 512],
                                start=(c == 0),
                                stop=(c == DC - 1),
                                skip_group_check=True,
                            )
                    for jh in range(4):
                        psum_copy(
                            v_sb[:, kc256 * 2 + jh // 2, (jh % 2) * 512 : (jh % 2 + 1) * 512],
                            vpss[jh][:],
                            jh,
                        )
                    return
                for j in range(2):
                    for h in range(2):
                        vps = aps.tile([128, 512], FP32, tag="ps", name="vps")
                        for c in range(DC):
                            nc.tensor.matmul(
                                vps[:],
                                xch[:, c, j * 128 : (j + 1) * 128],
                                wv_t[c][:, h * 512 : (h + 1) * 512],
                                start=(c == 0),
                                stop=(c == DC - 1),
                            )
                        psum_copy(
                            v_sb[:, kc256 * 2 + j, h * 512 : (h + 1) * 512], vps[:], j * 2 + h
                        )

            # A1: K^T[m, k] = sum_c W_K[c, m].T @ X^T[c, k]
            wk_t = []
            WK_SWDGE = cfg.get("wk_swdge", 1)
            for c in range(DC):
                w_tile = wp.tile([128, D], FP32R, name=f"wk{c}", tag="w")
                eng = nc.gpsimd if c < WK_SWDGE else nc.scalar
                eng.dma_start(w_tile[:], wk[c])
                wk_t.append(w_tile)
            with tc.tile_pool(name="xch1", bufs=2) as xp1:
                for kc in range(3):  # first three 512-key chunks
                    xch = xp1.tile([128, DC, 512], FP32R, tag="x1", name="xch1")
                    for c in range(DC):
                        nc.sync.dma_start(xch[:, c, :], xt512[kc, c])
                    if kc == 0:
                        # c-outer: first matmul needs only wk[0] + one X slice
                        kpss = [aps.tile([128, 512], FP32, tag="ps", name=f"kps{m}") for m in range(DC)]
                        for c in range(DC):
                            for m in range(DC):
                                nc.tensor.matmul(
                                    kpss[m][:],
                                    wk_t[c][:, m * 128 : (m + 1) * 128],
                                    xch[:, c, :],
                                    start=(c == 0),
                                    stop=(c == DC - 1),
                                    skip_group_check=True,
                                )
                        for m in range(DC):
                            psum_copy(
                                kt_sb[:, m, kc * 512 : (kc + 1) * 512], kpss[m][:], m
                            )
                    else:
                        for m in range(DC):
                            kps = aps.tile([128, 512], FP32, tag="ps")
                            for c in range(DC):
                                nc.tensor.matmul(
                                    kps[:],
                                    wk_t[c][:, m * 128 : (m + 1) * 128],
                                    xch[:, c, :],
                                    start=(c == 0),
                                    stop=(c == DC - 1),
                                )
                            psum_copy(
                                kt_sb[:, m, kc * 512 : (kc + 1) * 512], kps[:], m
                            )
                # last 512 keys as two 256-key chunks from the shared pool so
                # A2 can reuse the tiles without any phase-boundary DMA
                xt6_t = xsp.tile([128, DC, 256], FP32R, tag="x", name="xt6")
                nc.sync.dma_start(xt6_t[:], xt[6])
                kproj_256(xt6_t, 6)
                xt7_t = xsp.tile([128, DC, 256], FP32R, tag="x", name="xt7")
                nc.sync.dma_start(xt7_t[:], xt[7])
                if cfg.get("a1_tail_couter", True):
                    kproj_256_couter(xt7_t, 7)
                else:
                    kproj_256(xt7_t, 7)

            # A2: V[k, n] = sum_c X^T[c, k].T @ W_V[c, n]; reverse key order,
            # first two chunks reuse A1's resident tiles
            wv_t = []
            for c in range(DC):
                w_tile = wp.tile([128, D], FP32R, name=f"wv{c}", tag="w")
                nc.scalar.dma_start(w_tile[:], wv[c])
                wv_t.append(w_tile)
            stage_cm = tc.tile_pool(name="stage", bufs=cfg.get("stbufs", 3))
            stp = stage_cm.__enter__()
            xq3_t = None
            vproj_256(xt7_t, 7, couter=V_COUTER)
            vproj_256(xt6_t, 6)
            for kc in (5, 4, 3, 2, 1, 0):
                xch = xsp.tile([128, DC, 256], FP32R, tag="x", name="xch2")
                nc.sync.dma_start(xch[:], xt[kc])
                if kc == 5:
                    # prefetch A3's first query chunk while A2 still runs
                    xq3_t = stp.tile([128, DC, 256], FP32R, tag="qs", name="xq3")
                    nc.sync.dma_start(xq3_t[:], xtq[3])
                vproj_256(xch, kc, couter=(kc == 0 and cfg.get("a2_tail_couter", True)))

            # A3: Q^T[m, q] = (sum_c W_Q[c, m].T @ Xq^T[c, q]) * SCALE -> DRAM
            wq_t = []
            for c in range(DC):
                w_tile = wp.tile([128, D], FP32R, name=f"wq{c}", tag="w")
                nc.scalar.dma_start(w_tile[:], wq[c])
                wq_t.append(w_tile)
            if True:
                for qc in (3, 2, 1, 0):
                    if qc == 3:
                        xch = xq3_t
                    else:
                        xch = xsp.tile([128, DC, 256], FP32R, tag="x", name="xch3")
                        nc.sync.dma_start(xch[:], xtq[qc])
                    qstage = stp.tile([128, DC, 256], FP32R, tag="qs")
                    if qc == 3 and A3_COUTER:
                        qpss = [aps.tile([128, 512], FP32, tag="ps", name=f"qps{m}") for m in range(DC)]
                        for c in range(DC):
                            for m in range(DC):
                                nc.tensor.matmul(
                                    qpss[m][:, 0:256],
                                    wq_t[c][:, m * 128 : (m + 1) * 128],
                                    xch[:, c, :],
                                    start=(c == 0),
                                    stop=(c == DC - 1),
                                    skip_group_check=True,
                                )
                        for m in range(DC):
                            if cfg.get("q3_alt", False) and m % 2 == 1:
                                nc.vector.tensor_scalar_mul(
                                    qstage[:, m, :], qpss[m][:, 0:256], float(SCALE)
                                )
                            else:
                                nc.scalar.activation(qstage[:, m, :], qpss[m][:, 0:256], COPY, scale=float(SCALE))
                    else:
                        for m in range(DC):
                            qps = aps.tile([128, 512], FP32, tag="ps")
                            for c in range(DC):
                                nc.tensor.matmul(
                                    qps[:, 0:256],
                                    wq_t[c][:, m * 128 : (m + 1) * 128],
                                    xch[:, c, :],
                                    start=(c == 0),
                                    stop=(c == DC - 1),
                                )
                            nc.scalar.activation(qstage[:, m, :], qps[:, 0:256], COPY, scale=float(SCALE))
                    nc.scalar.dma_start(qt_dram[qc], qstage[:])
            stage_cm.__exit__(None, None, None)

        # ---------------- Phase B: attention ----------------
        with tc.tile_pool(name="qpool", bufs=1) as qp, \
             tc.tile_pool(name="ppool", bufs=cfg.get("pbufs", 2)) as pp, \
             tc.tile_pool(name="ptpool", bufs=cfg.get("ptbufs", 2)) as ptp, \
             tc.tile_pool(name="small", bufs=4) as smp, \
             tc.tile_pool(name="obuf", bufs=cfg.get("obbufs", 2)) as obp, \
             tc.tile_pool(name="spsum", bufs=cfg.get("sbufs", 3), space="PSUM") as sps, \
             tc.tile_pool(name="tpsum", bufs=cfg.get("tbufs", 1), space="PSUM") as tps, \
             tc.tile_pool(name="opsum", bufs=cfg.get("obufs", 2), space="PSUM") as ops:

            nc.sync.dma_start(ident[:], ident_in[:])
            nc.sync.dma_start(mask_sb[:], mask[:])
            qt_sb = qp.tile([128, DC, 4, 256], FP32R)
            if cfg.get("split_qt3", False):
                # split the critical first reload so t=7's S can start sooner
                nc.scalar.dma_start(qt_sb[:, 0:4, 3, :], qt_dram[3][:, 0:4, :])
                nc.scalar.dma_start(qt_sb[:, 4:8, 3, :], qt_dram[3][:, 4:8, :])
                for qc in (2, 1, 0):
                    nc.scalar.dma_start(qt_sb[:, :, qc, :], qt_dram[qc])
            else:
                for qc in (3, 2, 1, 0):
                    nc.scalar.dma_start(qt_sb[:, :, qc, :], qt_dram[qc])

            t_order = cfg.get("t_order", list(reversed(range(NQT))))
            for t in t_order:
                nk = (2 * t + 2) * 128
                nf, rem = divmod(nk, 512)
                widths = [512] * nf + ([rem] if rem else [])
                nch = len(widths)

                o_ps = ops.tile([128, D], FP32, tag="o")
                rsparts = smp.tile([128, 4], FP32, tag="rs")

                for ci, w in enumerate(widths):
                    kbase = ci * 512
                    last = ci == nch - 1
                    s_ps = sps.tile([128, 512], FP32, tag="s")
                    for c in range(DC):
                        nc.tensor.matmul(
                            s_ps[:, 0:w],
                            qt_sb[:, c, t // 2, (t % 2) * 128 : (t % 2) * 128 + 128],
                            kt_sb[:, c, kbase : kbase + w],
                            start=(c == 0),
                            stop=(c == DC - 1),
                        )
                    if last:
                        nc.vector.tensor_tensor(
                            s_ps[:, w - 256 : w], s_ps[:, w - 256 : w], mask_sb[:], ADD
                        )
                    p_sb = pp.tile([128, 512], FP32R, tag="p")
                    nc.scalar.activation(
                        p_sb[:, 0:w], s_ps[:, 0:w], EXP,
                        accum_out=rsparts[:, ci : ci + 1],
                    )
                    pt_ps = tps.tile([128, 512], FP32R, tag="pt")
                    for j in range(w // 128):
                        nc.tensor.transpose(
                            pt_ps[:, j * 128 : (j + 1) * 128],
                            p_sb[:, j * 128 : (j + 1) * 128],
                            ident[:],
                        )
                    pt_sb = ptp.tile([128, 512], FP32R, tag="pts")
                    nc.vector.tensor_copy(pt_sb[:, 0:w], pt_ps[:, 0:w])
                    for j in range(w // 128):
                        kt_idx = kbase // 128 + j
                        for h in range(2):
                            nc.tensor.matmul(
                                o_ps[:, h * 512 : (h + 1) * 512],
                                pt_sb[:, j * 128 : (j + 1) * 128],
                                v_sb[:, kt_idx, h * 512 : (h + 1) * 512],
                                start=(ci == 0 and j == 0),
                                stop=(last and j == w // 128 - 1),
                                skip_group_check=True,
                            )

                rs_tot = smp.tile([128, 1], FP32, tag="rst")
                nc.vector.tensor_reduce(rs_tot[:], rsparts[:, 0:nch], axis=AX, op=ADD)
                recip = smp.tile([128, 1], FP32, tag="rcp")
                nc.vector.reciprocal(recip[:], rs_tot[:])
                o_sb = obp.tile([128, D], FP32, tag="ob")
                for h in range(2):
                    nc.scalar.activation(
                        o_sb[:, h * 512 : (h + 1) * 512],
                        o_ps[:, h * 512 : (h + 1) * 512],
                        COPY,
                        scale=recip[:],
                    )
                    nc.sync.dma_start(o[t][:, h * 512 : (h + 1) * 512], o_sb[:, h * 512 : (h + 1) * 512])

    nc.compile()
    return nc


def _make_runner(nc):
    """Cached jitted 8-core runner (no donation; avoids per-call re-jit)."""
    import jax
    import numpy as np_
    from jax.sharding import Mesh, PartitionSpec
    from jax.experimental.shard_map import shard_map

    from concourse import mybir
    from concourse.bass2jax import (
        _bass_exec_p,
        install_neuronx_cc_hook,
        partition_id_tensor,
    )

    install_neuronx_cc_hook()
    partition_name = nc.partition_id_tensor.name if nc.partition_id_tensor else None
    in_names, out_names, out_avals = [], [], []
    for alloc in nc.m.functions[0].allocations:
        if not isinstance(alloc, mybir.MemoryLocationSet):
            continue
        name = alloc.memorylocations[0].name
        if alloc.kind == "ExternalInput":
            if name != partition_name:
                in_names.append(name)
        elif alloc.kind == "ExternalOutput":
            out_names.append(name)
            out_avals.append(
                jax.core.ShapedArray(
                    tuple(alloc.tensor_shape), mybir.dt.np(alloc.dtype)
                )
            )
    n_params = len(in_names)
    all_in = list(in_names) + list(out_names)
    if partition_name is not None:
        all_in.append(partition_name)

    def _body(*args):
        operands = list(args)
        if partition_name is not None:
            operands.append(partition_id_tensor())
        return tuple(
            _bass_exec_p.bind(
                *operands,
                out_avals=tuple(out_avals),
                in_names=tuple(all_in),
                out_names=tuple(out_names),
                lowering_input_output_aliases=(),
                sim_require_finite=True,
                sim_require_nnan=True,
                nc=nc,
            )
        )

    devices = jax.devices()[:NCORES]
    mesh = Mesh(np_.asarray(devices), ("core",))
    spec = PartitionSpec("core")
    fn = jax.jit(
        shard_map(
            _body,
            mesh=mesh,
            in_specs=(spec,) * (n_params + len(out_names)),
            out_specs=(spec,) * len(out_names),
            check_rep=False,
        ),
        keep_unused=True,
    )

    def run(in_maps):
        concat_in = [
            np_.concatenate([np_.asarray(m[nm]) for m in in_maps], axis=0)
            for nm in in_names
        ]
        zeros = [
            np_.zeros((NCORES * a.shape[0], *a.shape[1:]), a.dtype) for a in out_avals
        ]
        outs = fn(*concat_in, *zeros)
        return [
            {
                nm: np_.asarray(outs[i]).reshape(NCORES, *out_avals[i].shape)[c]
                for i, nm in enumerate(out_names)
            }
            for c in range(NCORES)
        ]

    return run


def kernel(embeddings, W_Q, W_K, W_V):
    from concourse.bass_utils import run_bass_kernel_spmd

    emb = np.ascontiguousarray(np.asarray(embeddings, dtype=np.float32))
    wq_np = np.ascontiguousarray(np.asarray(W_Q, dtype=np.float32)).reshape(DC, 128, D)
    wk_np = np.ascontiguousarray(np.asarray(W_K, dtype=np.float32)).reshape(DC, 128, D)
    wv_np = np.ascontiguousarray(np.asarray(W_V, dtype=np.float32)).reshape(DC, 128, D)

    if "nc" not in _CACHE:
        _CACHE["nc"] = _build()
    nc = _CACHE["nc"]

    tri = (np.arange(128)[:, None] >= np.arange(128)[None, :]).astype(np.float32)
    neg = np.float32(-1e9)
    masks = []
    for p in range(2):
        m = np.zeros((128, 256), dtype=np.float32)
        if p == 0:
            m[:, 0:128] = np.where(tri > 0, 0.0, neg)
            m[:, 128:256] = neg
        else:
            m[:, 0:128] = 0.0
            m[:, 128:256] = np.where(tri > 0, 0.0, neg)
        masks.append(m)
    ident_np = np.eye(128, dtype=np.float32)

    # per-batch X^T layouts (shared by the two cores of each batch)
    xt_b, xt512_b, xtb = [], [], []
    for b in range(B):
        x_t = np.ascontiguousarray(emb[b].T)  # [D, S]
        xtb.append(x_t)
        xt_b.append(
            np.ascontiguousarray(x_t.reshape(DC, 128, 8, 256).transpose(2, 1, 0, 3))
        )
        xt512_b.append(
            np.ascontiguousarray(x_t.reshape(DC, 128, 4, 512).transpose(2, 0, 1, 3))
        )

    in_maps = []
    for core in range(NCORES):
        b, p = divmod(core, 2)
        x_t = xtb[b]
        xt_np = xt_b[b]
        xt512_np = xt512_b[b]
        q_tiles = [x_t[:, (2 * t + p) * 128 : (2 * t + p + 1) * 128] for t in range(NQT)]
        xtq_np = np.concatenate(q_tiles, axis=1)  # [D, 1024]
        xtq_c = np.ascontiguousarray(
            xtq_np.reshape(DC, 128, 4, 256).transpose(2, 1, 0, 3)
        )
        in_maps.append(
            {
                "xt": xt_np,
                "xt512": xt512_np,
                "xtq": xtq_c,
                "wq": wq_np,
                "wk": wk_np,
                "wv": wv_np,
                "mask": masks[p],
                "ident": ident_np,
            }
        )

    global _last_in_maps
    _last_in_maps = in_maps
    results = None
    try:
        if "runner" not in _CACHE:
            _CACHE["runner"] = _make_runner(nc)
        results = _CACHE["runner"](in_maps)
    except Exception:
        _CACHE.pop("runner", None)
    if results is None:
        import time as _time

        for attempt in range(2):
            try:
                results = run_bass_kernel_spmd(
                    nc, in_maps, core_ids=list(range(NCORES))
                ).results
                break
            except Exception:
                if attempt == 1:
                    raise
                _time.sleep(3.0)

    out = np.empty((B, S, D), dtype=np.float32)
    for core in range(NCORES):
        b, p = divmod(core, 2)
        o_core = results[core]["o"]  # [NQT, 128, D]
        for t in range(NQT):
            gq = 2 * t + p
            out[b, gq * 128 : (gq + 1) * 128, :] = o_core[t]
    return out

